# revision 1
# baseline (speedup 1.0000x reference)
"""Trainium2 Bass kernel for nn_FSMNSeleNetV3 (FSMN stack + channel maxpool + decoder).

Self-contained: hardcodes all shapes from the problem spec and only imports
numpy + the concourse stack from /opt/trn_rl_repo.

Sharding: pure data parallel over batch. Each of the 8 cores processes 4
batches x 4 channels = 16 independent sequences of T=2048 tokens.

Layout: activations live as [feature_dim, time] in SBUF (features on
partitions). The host pre-transposes x to [B, C, F, T] so the DMA loads are
plain 2D loads. 64-channel tensors (shrink/FSMN stream) pack the two T/2
halves of a sequence onto 128 partitions so every engine sees full-width
tiles.

FSMN conv: 11 taps + residual. 8 taps + the residual identity run on the PE
as diagonal-matrix matmuls accumulating in PSUM (the two halves use disjoint
64x64 quadrants of the array, so they run concurrently); the remaining 3 taps
run as fused scalar_tensor_tensor FMAs on the DVE, with the first one reading
the conv PSUM directly as its addend. All matmuls use float32r (full-rate
fp32 streaming on the PE).
"""

import sys

sys.path.insert(0, "/opt/trn_rl_repo")
from contextlib import ExitStack

import numpy as np

import concourse.bass as bass  # noqa: F401
import concourse.mybir as mybir
import concourse.tile as tile
from concourse import bacc
from concourse.bass_utils import run_bass_kernel_spmd

F32 = mybir.dt.float32
F32R = mybir.dt.float32r
BF16 = mybir.dt.bfloat16
AF = mybir.ActivationFunctionType
OP = mybir.AluOpType

NCORES = 8
B, T, C, F = 32, 2048, 4, 120
DL, DP, L, LO, RO, S = 128, 64, 5, 10, 1, 5
BPC = B // NCORES  # batches per core
SEQ = BPC * C  # sequences per core
H = T // 2  # half-sequence length (halves stacked on partitions)
HALO_L = LO - 1  # 9 left halo columns
HW = H + HALO_L + RO  # h buffer width: 1034
NW = T // 512  # 512-token matmul windows per sequence

# conv tap split: delta in [-9..+1]; PE handles -9..-2 (+ residual identity),
# DVE handles -1, 0, +1 (tap table cols 8, 9, 10)
PE_DELTAS = list(range(-9, -1))  # 8 taps on the PE
NSLOT = len(PE_DELTAS) + 1  # + identity slot (always present, used when l>0)


def _mm(nc, out, lhsT, rhs, **kw):
    nc.tensor.matmul(out, lhsT, rhs, **kw)


def build_nc():
    nc = bacc.Bacc("TRN2", target_bir_lowering=False, debug=False, num_devices=NCORES)

    xt_d = nc.dram_tensor("xt", [SEQ, F, T], F32R, kind="ExternalInput")
    we0_d = nc.dram_tensor("we0", [F, DL], F32R, kind="ExternalInput")
    wedup_d = nc.dram_tensor("wedup", [L, 2 * DP, DL], F32R, kind="ExternalInput")
    ws_d = nc.dram_tensor("ws", [L, DL, DP], F32R, kind="ExternalInput")
    wd_d = nc.dram_tensor("wd", [DL, S], F32R, kind="ExternalInput")
    biases_d = nc.dram_tensor("biases", [DL, L + 1], F32, kind="ExternalInput")
    taps_d = nc.dram_tensor("taps", [2 * DP, L * 11], F32, kind="ExternalInput")
    diag_d = nc.dram_tensor("diag", [2 * DP, L * len(PE_DELTAS) * 2 * DP], BF16, kind="ExternalInput")
    ident_d = nc.dram_tensor("ident", [2 * DP, 2 * DP], F32R, kind="ExternalInput")
    bd_d = nc.dram_tensor("bd", [S, 1], F32, kind="ExternalInput")
    out_d = nc.dram_tensor("out", [BPC, S, T], F32, kind="ExternalOutput")

    with tile.TileContext(nc) as tc, ExitStack() as ctx:
        wp = ctx.enter_context(tc.tile_pool(name="weights", bufs=1))
        xp = ctx.enter_context(tc.tile_pool(name="x", bufs=3))
        ep = ctx.enter_context(tc.tile_pool(name="e", bufs=4))
        hp = ctx.enter_context(tc.tile_pool(name="h", bufs=4))
        op_ = ctx.enter_context(tc.tile_pool(name="o", bufs=4))
        fp = ctx.enter_context(tc.tile_pool(name="f", bufs=5))
        pp = ctx.enter_context(tc.tile_pool(name="pooled", bufs=2))
        osb = ctx.enter_context(tc.tile_pool(name="osb", bufs=2))
        ps = ctx.enter_context(tc.tile_pool(name="ps", bufs=4, space="PSUM"))
        psh = ctx.enter_context(tc.tile_pool(name="psh", bufs=4, space="PSUM"))

        # --- weights / constants (loaded once) ---
        we0_sb = wp.tile([F, DL], F32R)
        nc.sync.dma_start(out=we0_sb[:], in_=we0_d[:])
        wedup_sb = wp.tile([2 * DP, L * DL], F32R)
        ws_sb = wp.tile([DL, L * DP], F32R)
        for l in range(L):
            nc.sync.dma_start(out=wedup_sb[:, l * DL : (l + 1) * DL], in_=wedup_d[l])
            nc.sync.dma_start(out=ws_sb[:, l * DP : (l + 1) * DP], in_=ws_d[l])
        wd_sb = wp.tile([DL, S], F32R)
        nc.sync.dma_start(out=wd_sb[:], in_=wd_d[:])
        bias_sb = wp.tile([DL, L + 1], F32)
        nc.sync.dma_start(out=bias_sb[:], in_=biases_d[:])
        taps_sb = wp.tile([2 * DP, L * 11], F32)
        nc.sync.dma_start(out=taps_sb[:], in_=taps_d[:])
        diag_sb = wp.tile([2 * DP, L * len(PE_DELTAS) * 2 * DP], BF16)
        nc.sync.dma_start(out=diag_sb[:], in_=diag_d[:])
        ident_sb = wp.tile([2 * DP, 2 * DP], F32R)
        nc.sync.dma_start(out=ident_sb[:], in_=ident_d[:])
        bd_sb = wp.tile([S, 1], F32)
        nc.sync.dma_start(out=bd_sb[:], in_=bd_d[:])
        zero_sb = wp.tile([2 * DP, HALO_L], F32)
        nc.gpsimd.memset(zero_sb[:], 0.0)

        def tap(l, j):
            return taps_sb[:, l * 11 + j : l * 11 + j + 1]

        def diag(l, s):
            col = (l * len(PE_DELTAS) + s) * 2 * DP
            return diag_sb[:, col : col + 2 * DP]

        for b in range(BPC):
            f_tiles = []
            for c in range(C):
                seq = b * C + c

                x_sb = xp.tile([F, T], F32R)
                nc.sync.dma_start(out=x_sb[:], in_=xt_d[seq])

                # ---- unit-0 expand: relu(x @ We0 + be0), K=120 ----
                e_sb = ep.tile([DL, T], F32R)
                for w in range(NW):
                    pe = ps.tile([DL, 512], F32, tag="ps")
                    _mm(nc, pe[:], we0_sb[:], x_sb[:, w * 512 : (w + 1) * 512])
                    nc.scalar.activation(
                        e_sb[:, w * 512 : (w + 1) * 512],
                        pe[:],
                        AF.Relu,
                        bias=bias_sb[:, 0:1],
                        scale=1.0,
                    )

                o_prev = None
                for l in range(L):
                    if l > 0:
                        # ---- expand l: relu(o @ We[l-1] + be[l]), K=64,
                        # halves row-tiled concurrently ----
                        e_sb = ep.tile([DL, T], F32R)
                        for half in range(2):
                            q = half * DP
                            lhsT = wedup_sb[q : q + DP, (l - 1) * DL : l * DL]
                            for w in range(2):
                                pe = ps.tile([DL, 512], F32, tag="ps")
                                _mm(
                                    nc,
                                    pe[:],
                                    lhsT,
                                    o_prev[q : q + DP, w * 512 : (w + 1) * 512],
                                    tile_position=(q, 0),
                                )
                                col = (half * 2 + w) * 512
                                nc.scalar.activation(
                                    e_sb[:, col : col + 512],
                                    pe[:],
                                    AF.Relu,
                                    bias=bias_sb[:, l : l + 1],
                                    scale=1.0,
                                )

                    # ---- shrink l: h = e @ Ws[l], halves stacked into one
                    # PSUM bank via col tiling ----
                    h_sb = hp.tile([2 * DP, HW], BF16)
                    ws_l = ws_sb[:, l * DP : (l + 1) * DP]
                    phs = []
                    for w in range(2):
                        dst = slice(HALO_L + w * 512, HALO_L + (w + 1) * 512)
                        pha = psh.tile([DP, 512], F32, tag="ph")
                        phb = psh.tile([DP, 512], F32, tag="ph")
                        _mm(nc, pha[:], ws_l, e_sb[:, w * 512 : (w + 1) * 512])
                        _mm(nc, phb[:], ws_l, e_sb[:, H + w * 512 : H + (w + 1) * 512])
                        nc.vector.tensor_copy(h_sb[0:DP, dst], pha[:])
                        nc.scalar.copy(h_sb[DP : 2 * DP, dst], phb[:])
                        phs.append((pha, phb))

                    # ---- halo columns ----
                    nc.vector.tensor_copy(h_sb[0:DP, 0:HALO_L], zero_sb[0:DP, :])
                    nc.vector.tensor_copy(h_sb[DP : 2 * DP, H + HALO_L : HW], zero_sb[DP : 2 * DP, 0:RO])
                    nc.vector.tensor_copy(
                        h_sb[DP : 2 * DP, 0:HALO_L], h_sb[0:DP, H : H + HALO_L]
                    )
                    nc.vector.tensor_copy(
                        h_sb[0:DP, H + HALO_L : HW],
                        h_sb[DP : 2 * DP, HALO_L : HALO_L + 1],
                    )

                    # ---- FSMN conv ----
                    # PE part: 8 far-left taps (+ residual identity if l>0) as
                    # diagonal matmuls; halves in disjoint 64x64 quadrants.
                    pcs = []
                    for w in range(2):
                        pc = ps.tile([2 * DP, 512], F32, tag="ps")
                        nmm = len(PE_DELTAS) + (1 if l > 0 else 0)
                        i = 0
                        if l > 0:
                            _mm(
                                nc,
                                pc[:],
                                ident_sb[:],
                                o_prev[:, w * 512 : (w + 1) * 512],
                                start=True,
                                stop=(nmm == 1),
                            )
                            i = 1
                        for s, d in enumerate(PE_DELTAS):
                            a = HALO_L + d + w * 512
                            _mm(
                                nc,
                                pc[:],
                                diag(l, s),
                                h_sb[:, a : a + 512],
                                start=(i == 0),
                                stop=(i == nmm - 1),
                            )
                            i += 1
                        pcs.append(pc)

                    # DVE part (one PSUM operand per op): seed with
                    # delta=-1 adding the PE conv PSUM, then delta=0 from the
                    # exact fp32 shrink PSUM, then delta=+1 from bf16 h.
                    o_new = op_.tile([2 * DP, H], F32R)
                    for w in range(2):
                        ws_ = slice(w * 512, (w + 1) * 512)
                        a = HALO_L - 1 + w * 512
                        nc.vector.scalar_tensor_tensor(
                            o_new[:, ws_],
                            h_sb[:, a : a + 512],
                            tap(l, 8),
                            pcs[w][:],
                            OP.mult,
                            OP.add,
                        )
                    for w in range(2):
                        ws_ = slice(w * 512, (w + 1) * 512)
                        pha, phb = phs[w]
                        t9 = tap(l, 9)
                        nc.vector.scalar_tensor_tensor(
                            o_new[0:DP, ws_],
                            pha[:],
                            t9[0:DP, :],
                            o_new[0:DP, ws_],
                            OP.mult,
                            OP.add,
                        )
                        nc.vector.scalar_tensor_tensor(
                            o_new[DP : 2 * DP, ws_],
                            phb[:],
                            t9[DP : 2 * DP, :],
                            o_new[DP : 2 * DP, ws_],
                            OP.mult,
                            OP.add,
                        )
                    nc.vector.scalar_tensor_tensor(
                        o_new[:],
                        h_sb[:, HALO_L + 1 : HALO_L + 1 + H],
                        tap(l, 10),
                        o_new[:],
                        OP.mult,
                        OP.add,
                    )
                    o_prev = o_new

                # ---- final expand: relu(o @ We2 + be2) ----
                f_sb = fp.tile([DL, T], F32R)
                for half in range(2):
                    q = half * DP
                    lhsT = wedup_sb[q : q + DP, 4 * DL : 5 * DL]
                    for w in range(2):
                        pe = ps.tile([DL, 512], F32, tag="ps")
                        _mm(
                            nc,
                            pe[:],
                            lhsT,
                            o_prev[q : q + DP, w * 512 : (w + 1) * 512],
                            tile_position=(q, 0),
                        )
                        col = (half * 2 + w) * 512
                        nc.scalar.activation(
                            f_sb[:, col : col + 512],
                            pe[:],
                            AF.Relu,
                            bias=bias_sb[:, L : L + 1],
                            scale=1.0,
                        )
                f_tiles.append(f_sb)

            # ---- channel maxpool + decoder for this batch ----
            pooled = pp.tile([DL, T], F32R)
            nc.vector.tensor_max(pooled[:], f_tiles[0][:], f_tiles[1][:])
            nc.vector.tensor_max(pooled[:], pooled[:], f_tiles[2][:])
            nc.vector.tensor_max(pooled[:], pooled[:], f_tiles[3][:])

            out_sb = osb.tile([S, T], F32)
            for w in range(NW):
                pd = ps.tile([S, 512], F32, tag="ps")
                _mm(nc, pd[:], wd_sb[:], pooled[:, w * 512 : (w + 1) * 512])
                nc.scalar.activation(
                    out_sb[:, w * 512 : (w + 1) * 512],
                    pd[:],
                    AF.Identity,
                    bias=bd_sb[:, 0:1],
                    scale=1.0,
                )
            nc.sync.dma_start(out=out_d[b], in_=out_sb[:])

    nc.compile()
    return nc


_NC = None


def get_nc():
    global _NC
    if _NC is None:
        _NC = build_nc()
    return _NC


def prep_in_maps(x, We0, be0, Ws0, wl0, wr0, We, be, Ws, wl, wr, We2, be2, Wd, bd):
    xt = np.ascontiguousarray(x.transpose(0, 2, 3, 1), dtype=np.float32)  # [B,C,F,T]

    wedup = np.stack(
        [np.concatenate([w, w], axis=0) for w in [We[0], We[1], We[2], We[3], We2]]
    ).astype(np.float32)  # [L, 128, 128]
    ws_all = np.stack([Ws0, Ws[0], Ws[1], Ws[2], Ws[3]]).astype(np.float32)
    biases = np.stack([be0, be[0], be[1], be[2], be[3], be2], axis=1).astype(
        np.float32
    )  # [128, 6]

    wl_full = np.concatenate([wl0[None], wl], axis=0)  # [L, 10, 64]
    wr_full = np.concatenate([wr0[None], wr], axis=0)  # [L, 1, 64]
    taps64 = np.concatenate([wl_full, wr_full], axis=1).copy()  # [L, 11, 64]
    taps64[:, LO - 1, :] += 1.0  # conv identity term (o = h + left + right)
    taps = np.tile(
        taps64.transpose(2, 0, 1).reshape(DP, L * 11), (2, 1)
    )  # [128, 55], col = l*11 + j
    taps = np.ascontiguousarray(taps, dtype=np.float32)

    # diagonal tap matrices for the PE conv: slots 0..7 = taps delta=-9..-2
    # (tap table cols 0..7), slot 8 = identity (residual)
    import ml_dtypes

    npe = len(PE_DELTAS)
    diag = np.zeros((L, npe, 2 * DP, 2 * DP), np.float32)
    for l in range(L):
        for s in range(npe):
            np.fill_diagonal(diag[l, s], np.tile(taps64[l, s, :], 2))
    diag2 = diag.transpose(2, 0, 1, 3).reshape(2 * DP, L * npe * 2 * DP)
    diag2 = np.ascontiguousarray(diag2).astype(ml_dtypes.bfloat16)
    ident = np.eye(2 * DP, dtype=np.float32)

    shared = dict(
        we0=np.ascontiguousarray(We0, dtype=np.float32),
        wedup=wedup,
        ws=ws_all,
        wd=np.ascontiguousarray(Wd, dtype=np.float32),
        biases=np.ascontiguousarray(biases),
        taps=taps,
        diag=diag2,
        ident=ident,
        bd=np.ascontiguousarray(bd.reshape(S, 1), dtype=np.float32),
    )
    in_maps = []
    for k in range(NCORES):
        xs = xt[k * BPC : (k + 1) * BPC].reshape(SEQ, F, T)
        m = dict(shared)
        m["xt"] = np.ascontiguousarray(xs)
        in_maps.append(m)
    return in_maps


def postprocess(results):
    full = np.concatenate([r["out"] for r in results], axis=0)  # [B, S, T]
    return np.ascontiguousarray(full.transpose(0, 2, 1))  # [B, T, S]


def kernel(**inputs):
    nc = get_nc()
    in_maps = prep_in_maps(**inputs)
    res = run_bass_kernel_spmd(nc, in_maps, core_ids=list(range(NCORES)))
    return postprocess(res.results)



# revision 24
# speedup vs baseline: 1.3716x; 1.3716x over previous
"""Trainium2 Bass kernel for nn_FSMNSeleNetV3 (FSMN stack + channel maxpool + decoder).

Self-contained: hardcodes all shapes from the problem spec and only imports
numpy + the concourse stack from /opt/trn_rl_repo.

Sharding: pure data parallel over batch. Each of the 8 cores processes 4
batches x 4 channels = 16 independent sequences of T=2048 tokens.

v2 design (fp16 end-to-end):
- Sequences processed in PAIRS stacked on the 128 partitions (seq A channels
  on partitions 0..63, seq B on 64..127) so every DVE/ACT op runs full-width.
- All activations fp16 (numerically verified: rel_fro ~6e-4). DVE ops hit
  the 2x/4x perf modes (16-bit, step 1, 4B-aligned).
- FSMN unit: expand matmuls (row-split, concurrent PE quadrants) -> relu on
  ACT (free bias) -> shrink matmuls accumulate into a PSUM tile that the conv
  taps and the residual identity matmul then accumulate ONTO (fused
  shrink+conv+residual in PSUM). 6 taps run on the PE as block-diag matmuls;
  5 odd-shift taps run on DVE as tensor_scalar(4x) + tensor_tensor(2x) on the
  copied h. Final assembly o = psum + d in one DVE tensor_tensor per granule.
- h halo zeros live inside the h tile ([9 left | 2048 | 1 right]), set by two
  tiny ACT memzeros; no per-layer halo copies.
"""

import sys

sys.path.insert(0, "/opt/trn_rl_repo")
from contextlib import ExitStack

import numpy as np

import concourse.bass as bass  # noqa: F401
import concourse.mybir as mybir
import concourse.tile as tile
from concourse import bacc
from concourse.bass_utils import run_bass_kernel_spmd

F32 = mybir.dt.float32
F16 = mybir.dt.float16
AF = mybir.ActivationFunctionType
OP = mybir.AluOpType

NCORES = 8
B, T, C, F = 32, 2048, 4, 120
DL, DP, L, LO, RO, S = 128, 64, 5, 10, 1, 5
BPC = B // NCORES  # batches per core (4)
SEQ = BPC * C  # sequences per core (16)
NPAIR = SEQ // 2  # 8
G = 1024  # granule (PSUM tile width, fp32 -> 2 banks)
W = 512  # matmul window (one PSUM bank)
HW_ = LO - 1 + T + RO  # h tile width: 9 + 2048 + 1 = 2058

# conv tap split by delta (delta in [-9..+1]); j = delta + 9
PE_DELTAS = [-8, -6, -4, -2, 0, 1]  # block-diag matmuls on the PE
DVE_DELTAS = [-9, -7, -5, -3, -1]  # odd deltas -> 4-byte aligned fp16 reads
NPE = len(PE_DELTAS)
NDVE = len(DVE_DELTAS)

DEBUG = False  # adds dram dumps of pair-0/layer-0 intermediates
NPAIR_RUN = NPAIR  # debug knob: process fewer pairs
NL_RUN = L  # debug knob: run fewer FSMN layers
DECODE_RUN = True  # debug knob: disable maxpool+decode block


def build_nc():
    nc = bacc.Bacc("TRN2", target_bir_lowering=False, debug=False, num_devices=NCORES)

    xt_d = nc.dram_tensor("xt", [SEQ, F, T], F16, kind="ExternalInput")
    we0_d = nc.dram_tensor("we0", [F, DL], F16, kind="ExternalInput")
    # wed[k] = [We_k ; We_k] duplicated rows, k=0..3 layers 1..4, k=4 = We2
    wed_d = nc.dram_tensor("wed", [L, DL, DL], F16, kind="ExternalInput")
    # wsd[l,0] = blockdiag(Ws.lo, Ws.lo); wsd[l,1] = anti-blockdiag(Ws.hi, Ws.hi)
    wsd_d = nc.dram_tensor("wsd", [L, 2, DL, DL], F16, kind="ExternalInput")
    diag_d = nc.dram_tensor("diag", [DL, L * NPE * DL], F16, kind="ExternalInput")
    tapd_d = nc.dram_tensor("tapd", [DL, L * NDVE], F32, kind="ExternalInput")
    # bias2[:, 2l] = tile(be_l[:64], 2); bias2[:, 2l+1] = tile(be_l[64:], 2)
    bias2_d = nc.dram_tensor("bias2", [DL, 2 * L], F32, kind="ExternalInput")
    biasf_d = nc.dram_tensor("biasf", [DL, 1], F32, kind="ExternalInput")
    ident_d = nc.dram_tensor("ident", [DL, DL], F16, kind="ExternalInput")
    wd_d = nc.dram_tensor("wd", [DL, S], F16, kind="ExternalInput")
    bd_d = nc.dram_tensor("bd", [S, 1], F32, kind="ExternalInput")
    out_d = nc.dram_tensor("out", [BPC, S, T], F32, kind="ExternalOutput")
    if DEBUG:
        dbg_elo_d = nc.dram_tensor("dbg_elo", [DL, T], F16, kind="ExternalOutput")
        dbg_ehi_d = nc.dram_tensor("dbg_ehi", [DL, T], F16, kind="ExternalOutput")
        dbg_h_d = nc.dram_tensor("dbg_h", [DL, HW_], F16, kind="ExternalOutput")
        dbg_dd_d = nc.dram_tensor("dbg_dd", [DL, T], F16, kind="ExternalOutput")
        dbg_o_d = nc.dram_tensor("dbg_o", [DL, T], F16, kind="ExternalOutput")

    with tile.TileContext(nc) as tc, ExitStack() as ctx:
        wp = ctx.enter_context(tc.tile_pool(name="weights", bufs=1))
        xp = ctx.enter_context(tc.tile_pool(name="x", bufs=3))
        ep = ctx.enter_context(tc.tile_pool(name="e", bufs=3))
        hp = ctx.enter_context(tc.tile_pool(name="h", bufs=2))
        dp_ = ctx.enter_context(tc.tile_pool(name="d", bufs=2))
        op_ = ctx.enter_context(tc.tile_pool(name="o", bufs=3))
        fp = ctx.enter_context(tc.tile_pool(name="f", bufs=6))
        pp = ctx.enter_context(tc.tile_pool(name="pool", bufs=2))
        osb_p = ctx.enter_context(tc.tile_pool(name="osb", bufs=2))
        eps = ctx.enter_context(tc.tile_pool(name="eps", bufs=1, space="PSUM"))
        mps = ctx.enter_context(tc.tile_pool(name="mps", bufs=2, space="PSUM"))

        # ---- weights / constants (loaded once) ----
        we0_sb = wp.tile([F, DL], F16)
        nc.sync.dma_start(out=we0_sb[:], in_=we0_d[:])
        wed_sb = wp.tile([DL, L * DL], F16)
        wsd_sb = wp.tile([DL, L * 2 * DL], F16)
        for l in range(L):
            nc.sync.dma_start(out=wed_sb[:, l * DL : (l + 1) * DL], in_=wed_d[l])
            nc.sync.dma_start(
                out=wsd_sb[:, (2 * l) * DL : (2 * l + 1) * DL], in_=wsd_d[l, 0]
            )
            nc.sync.dma_start(
                out=wsd_sb[:, (2 * l + 1) * DL : (2 * l + 2) * DL], in_=wsd_d[l, 1]
            )
        diag_sb = wp.tile([DL, L * NPE * DL], F16)
        nc.sync.dma_start(out=diag_sb[:], in_=diag_d[:])
        tapd_sb = wp.tile([DL, L * NDVE], F32)
        nc.sync.dma_start(out=tapd_sb[:], in_=tapd_d[:])
        bias2_sb = wp.tile([DL, 2 * L], F32)
        nc.sync.dma_start(out=bias2_sb[:], in_=bias2_d[:])
        biasf_sb = wp.tile([DL, 1], F32)
        nc.sync.dma_start(out=biasf_sb[:], in_=biasf_d[:])
        ident_sb = wp.tile([DL, DL], F16)
        nc.sync.dma_start(out=ident_sb[:], in_=ident_d[:])
        wd_sb = wp.tile([DL, S], F16)
        nc.sync.dma_start(out=wd_sb[:], in_=wd_d[:])
        bd_sb = wp.tile([S, 1], F32)
        nc.sync.dma_start(out=bd_sb[:], in_=bd_d[:])

        def diag_ap(l, k):
            c = (l * NPE + k) * DL
            return diag_sb[:, c : c + DL]

        def tapd_ap(l, i):
            c = l * NDVE + i
            return tapd_sb[:, c : c + 1]

        f_tiles = []  # final-unit outputs for the current batch (4 seqs)

        for p in range(NPAIR_RUN):
            # ---- load the two sequences of this pair ----
            xa = xp.tile([F, T], F16, tag="x")
            xb = xp.tile([F, T], F16, tag="x")
            nc.sync.dma_start(out=xa[:], in_=xt_d[2 * p])
            nc.sync.dma_start(out=xb[:], in_=xt_d[2 * p + 1])

            o_prev = None
            for l in list(range(NL_RUN)) + [L]:  # l = L is the final expand
                if l == L:
                    # ---- final expand, per-seq full-width (feeds the maxpool) ----
                    f_a = fp.tile([DL, T], F16, tag="f", name="f_a")
                    f_b = fp.tile([DL, T], F16, tag="f", name="f_b")
                    lhs_base = (L - 1) * DL
                    for s, f_s in enumerate((f_a, f_b)):
                        q = s * DP
                        for g in range(2):
                            pf_t = eps.tile([DL, G], F32, tag="elo" if s == 0 else "ehi")
                            for w in range(2):
                                col = g * G + w * W
                                nc.tensor.matmul(
                                    pf_t[:, w * W : (w + 1) * W],
                                    wed_sb[q : q + DP, lhs_base : lhs_base + DL],
                                    o_prev[q : q + DP, col : col + W],
                                    start=True,
                                    stop=True,
                                    tile_position=(q, 0),
                                )
                            nc.scalar.activation(
                                f_s[:, g * G : (g + 1) * G],
                                pf_t[:],
                                AF.Relu,
                                bias=biasf_sb[:, 0:1],
                                scale=1.0,
                            )
                    f_tiles.append((f_a, f_b))
                    break

                # ---- expand + relu into pair-merged layout:
                #      e_lo = [A.lo; B.lo], e_hi = [B.hi; A.hi] (lane-aligned swap)
                e_lo = ep.tile([DL, T], F16, tag="elo_sb")
                e_hi = ep.tile([DL, T], F16, tag="ehi_sb")
                for g in range(2):
                    pl_t = eps.tile([DL, G], F32, tag="elo")
                    ph_t = eps.tile([DL, G], F32, tag="ehi")
                    for w in range(2):
                        col = g * G + w * W
                        dl = pl_t[:, w * W : (w + 1) * W]
                        dh = ph_t[:, w * W : (w + 1) * W]
                        if l == 0:
                            # K=120 single matmuls from x
                            nc.tensor.matmul(
                                dl[0:DP, :], we0_sb[:, 0:DP],
                                xa[:, col : col + W],
                                start=True, stop=True, tile_position=(0, 0),
                            )
                            nc.tensor.matmul(
                                dl[DP:DL, :], we0_sb[:, 0:DP],
                                xb[:, col : col + W],
                                start=True, stop=True, tile_position=(0, 64),
                            )
                            nc.tensor.matmul(
                                dh[DP:DL, :], we0_sb[:, DP:DL],
                                xa[:, col : col + W],
                                start=True, stop=True, tile_position=(0, 64),
                            )
                            nc.tensor.matmul(
                                dh[0:DP, :], we0_sb[:, DP:DL],
                                xb[:, col : col + W],
                                start=True, stop=True, tile_position=(0, 0),
                            )
                        else:
                            base = (l - 1) * DL
                            nc.tensor.matmul(
                                dl[0:DP, :], wed_sb[0:DP, base : base + DP],
                                o_prev[0:DP, col : col + W],
                                start=True, stop=True, tile_position=(0, 0),
                            )
                            nc.tensor.matmul(
                                dl[DP:DL, :], wed_sb[DP:DL, base : base + DP],
                                o_prev[DP:DL, col : col + W],
                                start=True, stop=True, tile_position=(64, 64),
                            )
                            nc.tensor.matmul(
                                dh[DP:DL, :], wed_sb[0:DP, base + DP : base + DL],
                                o_prev[0:DP, col : col + W],
                                start=True, stop=True, tile_position=(0, 64),
                            )
                            nc.tensor.matmul(
                                dh[0:DP, :], wed_sb[DP:DL, base + DP : base + DL],
                                o_prev[DP:DL, col : col + W],
                                start=True, stop=True, tile_position=(64, 0),
                            )
                    nc.scalar.activation(
                        e_lo[:, g * G : (g + 1) * G], pl_t[:],
                        AF.Relu, bias=bias2_sb[:, 2 * l : 2 * l + 1], scale=1.0,
                    )
                    nc.scalar.activation(
                        e_hi[:, g * G : (g + 1) * G], ph_t[:],
                        AF.Relu, bias=bias2_sb[:, 2 * l + 1 : 2 * l + 2], scale=1.0,
                    )

                # ---- shrink into PSUM (full-width groups), then conv+resid ----
                h = hp.tile([DL, HW_], F16, tag="h")
                nc.gpsimd.memset(h[:, 0 : LO - 1], 0.0)
                nc.gpsimd.memset(h[:, LO - 1 + T : HW_], 0.0)

                m_tiles = []
                for g in range(2):
                    mp_t = mps.tile([DL, G], F32, tag="m")
                    for w in range(2):
                        col = g * G + w * W
                        dst = mp_t[:, w * W : (w + 1) * W]
                        nc.tensor.matmul(
                            dst,
                            wsd_sb[:, (2 * l) * DL : (2 * l + 1) * DL],
                            e_lo[:, col : col + W],
                            start=True,
                            stop=False,
                        )
                        nc.tensor.matmul(
                            dst,
                            wsd_sb[:, (2 * l + 1) * DL : (2 * l + 2) * DL],
                            e_hi[:, col : col + W],
                            start=False,
                            stop=True,  # close the group so the h copy may read;
                            # the taps below keep accumulating (has_written stays
                            # set on HW; sim checks skipped via skip_group_check)
                        )
                    # copy h out of PSUM before the taps accumulate on top
                    nc.scalar.copy(h[:, LO - 1 + g * G : LO - 1 + (g + 1) * G], mp_t[:])
                    m_tiles.append(mp_t)

                # PE taps (+ residual identity) accumulate onto the shrink PSUM
                for g in range(2):
                    mp_t = m_tiles[g]
                    for w in range(2):
                        dst = mp_t[:, w * W : (w + 1) * W]
                        base = LO - 1 + g * G + w * W
                        for k, dlt in enumerate(PE_DELTAS):
                            a = base + dlt
                            last = (k == NPE - 1) and (l == 0)
                            nc.tensor.matmul(
                                dst,
                                diag_ap(l, k),
                                h[:, a : a + W],
                                start=False,
                                stop=last,
                                skip_group_check=True,
                            )
                        if l > 0:
                            nc.tensor.matmul(
                                dst,
                                ident_sb[:],
                                o_prev[:, g * G + w * W : g * G + (w + 1) * W],
                                start=False,
                                stop=True,
                                skip_group_check=True,
                            )

                # DVE taps: d = sum_j tap_j * h(shifted), fp16 4x/2x modes
                d = dp_.tile([DL, T], F16, tag="d")
                tmp = dp_.tile([DL, T], F16, tag="tmp")
                a0 = LO - 1 + DVE_DELTAS[0]
                nc.vector.tensor_scalar_mul(d[:], h[:, a0 : a0 + T], tapd_ap(l, 0))
                for i in range(1, NDVE):
                    a = LO - 1 + DVE_DELTAS[i]
                    nc.vector.tensor_scalar_mul(tmp[:], h[:, a : a + T], tapd_ap(l, i))
                    nc.vector.tensor_add(d[:], d[:], tmp[:])

                # o = conv_psum + d (one 1x tensor_tensor per granule)
                o_new = op_.tile([DL, T], F16, tag="o")
                for g in range(2):
                    nc.vector.tensor_add(
                        o_new[:, g * G : (g + 1) * G], m_tiles[g][:], d[:, g * G : (g + 1) * G]
                    )
                if DEBUG and p == 0 and l == 0:
                    nc.sync.dma_start(out=dbg_elo_d[:], in_=e_lo[:])
                    nc.sync.dma_start(out=dbg_ehi_d[:], in_=e_hi[:])
                    nc.sync.dma_start(out=dbg_h_d[:], in_=h[:])
                    nc.sync.dma_start(out=dbg_dd_d[:], in_=d[:])
                    nc.sync.dma_start(out=dbg_o_d[:], in_=o_new[:])
                o_prev = o_new

            # ---- per-batch maxpool + decode after the odd pair ----
            if p % 2 == 1 and DECODE_RUN:
                (f0a, f0b), (f1a, f1b) = f_tiles
                f_tiles = []
                mp1 = pp.tile([DL, T], F16, tag="mp")
                mp2 = pp.tile([DL, T], F16, tag="mp2")
                nc.vector.tensor_max(mp1[:], f0a[:], f0b[:])
                nc.vector.tensor_max(mp2[:], f1a[:], f1b[:])
                nc.vector.tensor_max(mp1[:], mp1[:], mp2[:])

                out_sb = osb_p.tile([S, T], F32, tag="out")
                for w in range(4):
                    pd = eps.tile([S, W], F32, tag="elo")
                    nc.tensor.matmul(
                        pd[:],
                        wd_sb[:],
                        mp1[:, w * W : (w + 1) * W],
                        start=True,
                        stop=True,
                    )
                    nc.scalar.activation(
                        out_sb[:, w * W : (w + 1) * W],
                        pd[:],
                        AF.Identity,
                        bias=bd_sb[:, 0:1],
                        scale=1.0,
                    )
                nc.sync.dma_start(out=out_d[p // 2], in_=out_sb[:])

    nc.compile()
    return nc


_NC = None


def get_nc():
    global _NC
    if _NC is None:
        _NC = build_nc()
    return _NC


def prep_in_maps(x, We0, be0, Ws0, wl0, wr0, We, be, Ws, wl, wr, We2, be2, Wd, bd):
    xt = np.ascontiguousarray(
        x.transpose(0, 2, 3, 1), dtype=np.float16
    )  # [B, C, F, T]

    wed = np.stack(
        [np.concatenate([w, w], axis=0) for w in [We[0], We[1], We[2], We[3], We2]]
    ).astype(np.float16)  # [5, 128, 128]

    # shrink weights: lo = blockdiag(Ws.lo, Ws.lo); hi = anti-blockdiag(Ws.hi)
    ws_list = [Ws0, Ws[0], Ws[1], Ws[2], Ws[3]]
    wsd = np.zeros((L, 2, DL, DL), np.float32)
    for l, w_ in enumerate(ws_list):
        wsd[l, 0, 0:DP, 0:DP] = w_[0:DP, :]
        wsd[l, 0, DP:DL, DP:DL] = w_[0:DP, :]
        wsd[l, 1, DP:DL, 0:DP] = w_[DP:DL, :]
        wsd[l, 1, 0:DP, DP:DL] = w_[DP:DL, :]
    wsd = wsd.astype(np.float16)

    # expand biases in pair-merged layout (lo: ch 0:64 twice, hi: ch 64:128 twice)
    be_list = [be0, be[0], be[1], be[2], be[3]]
    bias2 = np.zeros((DL, 2 * L), np.float32)
    for l, b_ in enumerate(be_list):
        bias2[:, 2 * l] = np.tile(b_[0:DP], 2)
        bias2[:, 2 * l + 1] = np.tile(b_[DP:DL], 2)
    biasf = np.ascontiguousarray(be2.reshape(DL, 1), dtype=np.float32)

    wl_full = np.concatenate([wl0[None], wl], axis=0)  # [5, 10, 64]
    wr_full = np.concatenate([wr0[None], wr], axis=0)  # [5, 1, 64]
    taps64 = np.concatenate([wl_full, wr_full], axis=1).copy()  # [5, 11, 64]
    # NOTE: no identity bump on the delta=0 tap — the shrink result already
    # sits in the conv PSUM as the identity contribution (o = h + left + right).

    # block-diag matrices for the PE taps: diag(tap twice) per (layer, slot)
    diag = np.zeros((L, NPE, 2 * DP, 2 * DP), np.float32)
    for l in range(L):
        for k, dlt in enumerate(PE_DELTAS):
            np.fill_diagonal(diag[l, k], np.tile(taps64[l, dlt + LO - 1, :], 2))
    diag2 = np.ascontiguousarray(
        diag.transpose(2, 0, 1, 3).reshape(2 * DP, L * NPE * 2 * DP)
    ).astype(np.float16)

    # per-partition scalars for the DVE taps
    tapd = np.zeros((2 * DP, L * NDVE), np.float32)
    for l in range(L):
        for i, dlt in enumerate(DVE_DELTAS):
            tapd[:, l * NDVE + i] = np.tile(taps64[l, dlt + LO - 1, :], 2)
    tapd = np.ascontiguousarray(tapd)

    shared = dict(
        we0=np.ascontiguousarray(We0, dtype=np.float16),
        wed=wed,
        wsd=np.ascontiguousarray(wsd),
        diag=diag2,
        tapd=tapd,
        bias2=np.ascontiguousarray(bias2),
        biasf=biasf,
        ident=np.eye(2 * DP, dtype=np.float16),
        wd=np.ascontiguousarray(Wd, dtype=np.float16),
        bd=np.ascontiguousarray(bd.reshape(S, 1), dtype=np.float32),
    )
    in_maps = []
    for k in range(NCORES):
        xs = xt[k * BPC : (k + 1) * BPC].reshape(SEQ, F, T)
        m = dict(shared)
        m["xt"] = np.ascontiguousarray(xs)
        in_maps.append(m)
    return in_maps


def postprocess(results):
    full = np.concatenate([r["out"] for r in results], axis=0)  # [B, S, T]
    return np.ascontiguousarray(full.transpose(0, 2, 1))  # [B, T, S]


def kernel(**inputs):
    nc = get_nc()
    in_maps = prep_in_maps(**inputs)
    res = run_bass_kernel_spmd(nc, in_maps, core_ids=list(range(NCORES)))
    return postprocess(res.results)
